# revision 47
# baseline (speedup 1.0000x reference)
"""Trainium2 Bass kernel for nn_NetworkLayer_79173427134941 (gnn_message_passing).

Reference computation (per batch item b, N=1024 points, 3D coords):
    norms = ||x_b||                      [N, 1]
    dots  = sqrt(x_b @ x_b^T)            [N, N]
    scalars = [u_b (G=8) | norms | dots] [N, 1033]
    h = LeakyReLU(scalars @ W0 + b0); h = LeakyReLU(h @ W1 + b1)
    fk = h @ W2 + b2                     [N, 128]
    out_b = einsum('io,id->od', fk, x_b) / N    [128, 3]

Strategy:
  - Data-parallel over batch: 4 batch items per core x 8 cores.
  - Never materialize dots in HBM: gram on TensorE, sqrt on ScalarE
    during PSUM->SBUF eviction, MLP fused on-chip in transposed [H, N] layout.
  - u-part + b0 folded into a host-precomputed K=2 rhs chunk [norms; ones].
  - LeakyReLU evictions fused to one DVE op each: leaky(x) = max(0.01x, x).
  - Final contraction y_b = x_b^T @ h1 runs as multiply-accumulate on the
    (otherwise idle) Pool engine; the last tiny [3,128]@[128,128] matmul +
    bias outer product runs on host:
       out_b^T = (x_b^T @ h1) @ W2 + b2 (x) colsum(x_b)
"""

import numpy as np

B, N, G = 32, 1024, 8
H, K_OUT = 128, 128
N_CORES = 8
BPC = B // N_CORES  # batch items per core

_cached = {}
PRECISION = "f16"


def _build_nc(precision=None, repeat=1, with_b1=True, fuse_leaky=False,
              y_engine="vector", interleave=True, leaky_wide=True,
              use_act_tail=False, pe_y=True, rotate=True, rowtile=False,
              wide_mm=False, abl=None):
    import concourse.tile as tile
    from concourse import bacc, mybir

    precision = precision or PRECISION
    f32 = mybir.dt.float32
    f32r = mybir.dt.float32r
    f16 = mybir.dt.float16
    tdt = f16 if precision == "f16" else f32r   # tail: h0/h1c/w1/xc
    mdt = f16 if precision == "f16" else f32r   # mid: dots/w0d
    MUL = mybir.AluOpType.mult
    ADD = mybir.AluOpType.add
    MAX = mybir.AluOpType.max

    nc = bacc.Bacc(
        "TRN2",
        target_bir_lowering=False,
        debug=False,
        enable_asserts=True,
        num_devices=N_CORES,
    )

    # DRAM I/O (per core)
    use_y_pre = abl not in ("no_y", "no_mlp", "empty")
    gdt = tdt
    xT_d = nc.dram_tensor("xT", [BPC, 3, N], gdt, kind="ExternalInput").ap()
    xr_d = None
    if use_y_pre and pe_y:
        xr_d = nc.dram_tensor("xr", [BPC, 128, 3 * (N // 128)], tdt,
                              kind="ExternalInput").ap()
    rhs2_d = nc.dram_tensor("rhs2", [BPC, 2, N], tdt, kind="ExternalInput").ap()
    lw2_d = nc.dram_tensor("lw2", [BPC, 2, H], tdt, kind="ExternalInput").ap()
    w0d_d = nc.dram_tensor("w0d", [128, 1024], mdt, kind="ExternalInput").ap()
    w1_d = nc.dram_tensor("w1", [128, H], tdt, kind="ExternalInput").ap()
    b1t_d = ones_d = None
    if with_b1:
        b1t_d = nc.dram_tensor("b1t", [1, N], tdt, kind="ExternalInput").ap()
        ones_d = nc.dram_tensor("ones", [1, N], tdt, kind="ExternalInput").ap()
    y_d = nc.dram_tensor("y", [BPC, H, 3], f32, kind="ExternalOutput").ap()

    NCHUNK = N // 128  # 8 K-chunks of the dots matmul
    use_y = abl not in ("no_y", "no_mlp", "empty")
    use_mlp = abl not in ("no_mlp", "empty")
    use_gram = abl != "empty"
    gram_strips = 1 if abl == "tiny_gram" else NCHUNK

    with tile.TileContext(nc) as tc:
        with (
            tc.tile_pool(name="const", bufs=1) as constp,
            tc.tile_pool(name="slots", bufs=1) as slotp,
            tc.tile_pool(name="data", bufs=2) as datap,
            tc.tile_pool(name="dots", bufs=2) as dotsp,
            tc.tile_pool(name="act", bufs=2) as actp,
            tc.tile_pool(name="yout", bufs=2) as youtp,
            tc.tile_pool(name="gram", bufs=2, space="PSUM") as gramp,
            tc.tile_pool(name="h0p", bufs=1, space="PSUM") as h0pp,
            tc.tile_pool(name="h1p", bufs=1, space="PSUM") as h1pp,
            tc.tile_pool(name="yp", bufs=1, space="PSUM") as ypp,
        ):
            # const tiles (DMAs issued after batch-0 loads; see emit_consts)
            w0d_sb = constp.tile([128, 1024], mdt)
            w1_sb = constp.tile([128, H], tdt)
            b1t_sb = ones_sb = None
            if with_b1:
                b1t_sb = constp.tile([1, N], tdt, name="b1t_sb")
                ones_sb = constp.tile([1, N], tdt, name="ones_sb")

            def emit_consts():
                if not use_mlp:
                    return
                nc.sync.dma_start(out=w0d_sb[:], in_=w0d_d[:])
                nc.sync.dma_start(out=w1_sb[:], in_=w1_d[:])
                if with_b1:
                    nc.sync.dma_start(out=b1t_sb[:], in_=b1t_d[:])
                    nc.sync.dma_start(out=ones_sb[:], in_=ones_d[:])

            def leaky_evict(out_ap, ps_ap, use_act=False):
                # leaky(x) = 0.01*x + 0.99*relu(x); two ops so each reads
                # PSUM once (walrus forbids two PSUM reads per instruction).
                ltmp = actp.tile(
                    [ps_ap.shape[0], ps_ap.shape[1]], f32, tag="ltmp", bufs=4
                )
                if use_act:
                    # Act has slack at the pipeline tail once sqrts drained
                    nc.scalar.activation(
                        ltmp[:], ps_ap, mybir.ActivationFunctionType.Relu,
                        bias=0.0, scale=0.99,
                    )
                else:
                    nc.vector.tensor_scalar(
                        ltmp[:], ps_ap, 0.0, 0.99, MAX, MUL
                    )
                nc.vector.scalar_tensor_tensor(
                    out_ap, ps_ap, 0.01, ltmp[:], MUL, ADD
                )

            def alloc_loads(b, st, persist=False):
                """Create the input + dots tiles for item b's slot."""
                P = slotp if persist else None
                sfx = str(b) if persist else ""

                def mk(pool, shape, dt_, tag):
                    pp = P if persist else pool
                    return pp.tile(shape, dt_, tag=tag + sfx, name=f"{tag}{b}")

                st["xT"] = mk(datap, [35, N] if rowtile else [3, N], gdt, "xT")
                if use_mlp:
                    st["rhs2"] = mk(datap, [2, N], tdt, "rhs2")
                    st["lw2"] = mk(datap, [2, H], tdt, "lw2")
                st["dots"] = mk(dotsp, [128, gram_strips * N], mdt, "dots")
                if use_y and pe_y:
                    # x chunk-rows [i%128, 3*(i//128)+d] for the PE y-reduce
                    st["xr"] = mk(datap, [128, 3 * NCHUNK], tdt, "xr")
                elif use_y:
                    # x^T rows, broadcast on-chip across partitions (Pool)
                    st["xf"] = mk(datap, [1, 3 * N], tdt, "xf")
                    st["xbc"] = mk(datap, [128, 3 * N], tdt, "xbc")

            def emit_loads(b, st):
                """Input DMAs for item b (prefetchable)."""
                if "xT" not in st:
                    alloc_loads(b, st)
                nc.sync.dma_start(out=st["xT"][0:3, :], in_=xT_d[b])
                if rowtile:
                    # second copy at partition 32 for PE row-group 1
                    nc.sync.dma_start(out=st["xT"][32:35, :], in_=xT_d[b])
                if use_mlp:
                    nc.sync.dma_start(out=st["rhs2"][:], in_=rhs2_d[b])
                    nc.sync.dma_start(out=st["lw2"][:], in_=lw2_d[b])
                if use_y and pe_y:
                    nc.sync.dma_start(out=st["xr"][:], in_=xr_d[b])
                elif use_y:
                    nc.sync.dma_start(out=st["xf"][:],
                                      in_=xT_d[b].flatten()[None, :])
                    nc.gpsimd.partition_broadcast(st["xbc"][:], st["xf"][:])

            def emit_gram_strip(b, m, st):
                """Gram strip m of batch b: 2 matmuls + sqrt eviction."""
                if m == 0 and "xT" not in st:
                    emit_loads(b, st)
                xT_sb, dots_sb = st["xT"], st["dots"]
                g_ps = gramp.tile([128, N], f32, tag="g", name=f"g{b}_{m}")
                # with rowtile, alternate strips between PE row groups 0/1
                # (base partition 0/32) so consecutive strips' matmuls run
                # concurrently in the array
                base = 32 * (m % 2) if rowtile else 0
                lhsT = xT_sb[base : base + 3, 128 * m : 128 * (m + 1)]
                if wide_mm:
                    nc.tensor.matmul(
                        g_ps[:], lhsT, xT_sb[base : base + 3, :],
                        start=True, stop=True,
                    )
                else:
                    for half in range(2):
                        nc.tensor.matmul(
                            g_ps[:, 512 * half : 512 * (half + 1)],
                            lhsT,
                            xT_sb[base : base + 3, 512 * half : 512 * (half + 1)],
                            start=True,
                            stop=True,
                        )
                nc.scalar.sqrt(dots_sb[:, N * m : N * (m + 1)], g_ps[:])

            def emit_h0_chunk(b, c, st):
                """Layer-0 K-chunk c of batch b (needs dots strip c only)."""
                if c == 0:
                    h0_ps = h0pp.tile([128, N], f32, tag="h0ps", name=f"h0ps{b}")
                    st["h0ps"] = h0_ps
                    if wide_mm:
                        nc.tensor.matmul(
                            h0_ps[:], st["lw2"][:], st["rhs2"][:],
                            start=True, stop=False,
                        )
                    else:
                        for half in range(2):
                            sl = slice(512 * half, 512 * (half + 1))
                            nc.tensor.matmul(
                                h0_ps[:, sl],
                                st["lw2"][:],
                                st["rhs2"][:, sl],
                                start=True,
                                stop=False,
                            )
                h0_ps, dots_sb = st["h0ps"], st["dots"]
                cc = min(c, gram_strips - 1)
                lhsT = w0d_sb[:, 128 * c : 128 * (c + 1)]
                if wide_mm:
                    nc.tensor.matmul(
                        h0_ps[:], lhsT, dots_sb[:, N * cc : N * (cc + 1)],
                        start=False, stop=(c == NCHUNK - 1),
                    )
                else:
                    for half in range(2):
                        nc.tensor.matmul(
                            h0_ps[:, 512 * half : 512 * (half + 1)],
                            lhsT,
                            dots_sb[:, N * cc + 512 * half : N * cc + 512 * (half + 1)],
                            start=False,
                            stop=(c == NCHUNK - 1),
                        )
                if c == NCHUNK - 1:
                    h0_sb = actp.tile([128, N], tdt, tag="h0", name=f"h0{b}")
                    st["h0"] = h0_sb
                    ua = use_act_tail and b == BPC - 1
                    if leaky_wide:
                        leaky_evict(h0_sb[:], h0_ps[:], use_act=ua)
                    else:
                        for half in range(2):
                            sl = slice(512 * half, 512 * (half + 1))
                            leaky_evict(h0_sb[:, sl], h0_ps[:, sl], use_act=ua)

            def emit_tail_pe_y(b, st):
                """Layer 1 in [N, H] layout (lhsT = h0 blocks, rhs = W1) so
                the output contraction y = h1T^T @ x runs on the PE as 8
                tiny accumulating matmuls — no xbc broadcast, no DVE y."""
                h0_sb = st["h0"]
                ua = use_act_tail and b == BPC - 1
                h1t_sb = actp.tile([128, N], tdt, tag="h1c", name=f"h1c{b}")
                for mg in range(2):
                    ps = h1pp.tile([128, 512], f32, tag="h1ps",
                                   name=f"h1ps{b}_{mg}")
                    for q in range(4):
                        m = 4 * mg + q
                        qsl = slice(128 * q, 128 * (q + 1))
                        if with_b1:
                            nc.tensor.matmul(
                                ps[:, qsl],
                                ones_sb[:, 0:128],
                                b1t_sb[:, 0:128],
                                start=True,
                                stop=False,
                            )
                        nc.tensor.matmul(
                            ps[:, qsl],
                            h0_sb[:, 128 * m : 128 * (m + 1)],
                            w1_sb[:],
                            start=not with_b1,
                            stop=True,
                        )
                    leaky_evict(h1t_sb[:, 512 * mg : 512 * (mg + 1)], ps[:],
                                use_act=ua)
                # y[h, d] = sum_i h1T[i, h] x[i, d]: lhsT = h1T block,
                # rhs = x chunk -> only 3 output columns per matmul
                xr_sb = st["xr"]
                yT_ps = ypp.tile([128, 4], f32, tag="yps", name=f"yps{b}")
                for m in range(NCHUNK):
                    nc.tensor.matmul(
                        yT_ps[:, 0:3],
                        h1t_sb[:, 128 * m : 128 * (m + 1)],
                        xr_sb[:, 3 * m : 3 * (m + 1)],
                        start=(m == 0),
                        stop=(m == NCHUNK - 1),
                    )
                yT_sb = youtp.tile([128, 4], f32, tag="y", name=f"y{b}")
                nc.vector.tensor_scalar(yT_sb[:, 0:3], yT_ps[:, 0:3], 1.0,
                                        None, MUL)
                nc.sync.dma_start(out=y_d[b], in_=yT_sb[:, 0:3])

            def emit_tail(b, st):
                """Layer 1 (transposed [H, N] layout) + output contraction."""
                if pe_y and use_y:
                    emit_tail_pe_y(b, st)
                    return
                h0_sb = st["h0"]
                h1_ps = h1pp.tile([128, N], f32, tag="h1ps", name=f"h1ps{b}")
                for half in range(2):
                    sl = slice(512 * half, 512 * (half + 1))
                    if with_b1:
                        nc.tensor.matmul(
                            h1_ps[:, sl],
                            b1t_sb[:, 0:128],
                            ones_sb[:, sl],
                            start=True,
                            stop=False,
                        )
                    nc.tensor.matmul(
                        h1_ps[:, sl],
                        w1_sb[:],
                        h0_sb[:, sl],
                        start=not with_b1,
                        stop=True,
                    )
                h1c_sb = actp.tile([128, N], tdt, tag="h1c", name=f"h1c{b}")
                ua = use_act_tail and b == BPC - 1
                if leaky_wide:
                    leaky_evict(h1c_sb[:], h1_ps[:], use_act=ua)
                else:
                    for half in range(2):
                        sl = slice(512 * half, 512 * (half + 1))
                        leaky_evict(h1c_sb[:, sl], h1_ps[:, sl], use_act=ua)

                yT_sb = youtp.tile([128, 4], f32, tag="y", name=f"y{b}")
                if use_y:
                    # y_b^T[h, d] = sum_i h1^T[h, i] * x[i, d]: free-axis
                    # multiply-reduce against the broadcast x rows
                    xbc_sb = st["xbc"]
                    yeng = nc.gpsimd if y_engine == "gpsimd" else nc.vector
                    for d in range(3):
                        ysc = actp.tile([128, N], tdt, tag="ysc",
                                        name=f"ysc{b}_{d}")
                        yeng.scalar_tensor_tensor(
                            ysc[:],
                            h1c_sb[:],
                            1.0,
                            xbc_sb[:, N * d : N * (d + 1)],
                            MUL,
                            MUL,
                            accum_out=yT_sb[:, d : d + 1],
                        )
                else:
                    nc.vector.tensor_scalar(
                        yT_sb[:, 0:3], h1c_sb[:, 0:3], 1.0, None, MUL
                    )
                nc.sync.dma_start(out=y_d[b], in_=yT_sb[:, 0:3])

            def emit_dummy_tail(b, st):
                """Ablation tails that keep the output DMA alive."""
                yT_sb = youtp.tile([128, 4], f32, tag="y", name=f"y{b}")
                if use_gram:
                    nc.vector.tensor_scalar(
                        yT_sb[:, 0:3], st["dots"][:, 0:3], 1.0, None, MUL
                    )
                else:
                    nc.vector.memset(yT_sb[:, 0:3], 0.0)
                nc.sync.dma_start(out=y_d[b], in_=yT_sb[:, 0:3])

            def emit_all():
                states = [dict() for _ in range(BPC)]
                if abl == "empty":
                    for b in range(BPC):
                        xT_sb = datap.tile([3, N], gdt, tag="xT", name=f"xT{b}")
                        nc.sync.dma_start(out=xT_sb[0:3, :], in_=xT_d[b])
                        states[b]["xT"] = xT_sb
                        emit_dummy_tail(b, states[b])
                    return
                if interleave and use_mlp and gram_strips == NCHUNK:
                    # Stage b emits gram strips of item b interleaved with h0
                    # chunks of item b-1, so the PE FIFO always has gram work
                    # to fill the sqrt-paced h0 stalls, and the sqrt stream of
                    # item b starts right behind item b-1's. Input DMAs are
                    # prefetched one stage ahead.
                    emit_loads(0, states[0])
                    emit_consts()
                    for b in range(BPC + 1):
                        if b + 1 < BPC:
                            emit_loads(b + 1, states[b + 1])
                        for m in range(NCHUNK):
                            if b < BPC:
                                emit_gram_strip(b, m, states[b])
                            if b >= 1:
                                emit_h0_chunk(b - 1, m, states[b - 1])
                        if b >= 1:
                            emit_tail(b - 1, states[b - 1])
                    return
                for b in range(BPC):
                    if b == 0:
                        emit_loads(0, states[0])
                        emit_consts()
                    for m in range(gram_strips):
                        emit_gram_strip(b, m, states[b])
                    if use_mlp:
                        if b >= 1:
                            emit_tail(b - 1, states[b - 1])
                        for c in range(NCHUNK):
                            emit_h0_chunk(b, c, states[b])
                    else:
                        emit_dummy_tail(b, states[b])
                if use_mlp:
                    emit_tail(BPC - 1, states[BPC - 1])

            can_rotate = rotate and interleave and use_mlp and gram_strips == NCHUNK
            if repeat == 1:
                emit_all()
            elif can_rotate:
                # Software-pipeline ACROSS loop iterations: each body stage
                # b runs gram(b) + h0(b-1 mod 4) + tail(b-1 mod 4); the
                # b=0 stage consumes slot 3 written by the previous
                # iteration (primed once by the prologue), so the sqrt
                # stream never drains at the loop back-edge.
                states = [dict() for _ in range(BPC)]
                for b in range(BPC):
                    alloc_loads(b, states[b], persist=True)
                emit_loads(BPC - 1, states[BPC - 1])
                emit_consts()  # loop-invariant: DMA'd once, outside the loop
                for m in range(NCHUNK):
                    emit_gram_strip(BPC - 1, m, states[BPC - 1])
                emit_loads(0, states[0])
                with tc.For_i(0, repeat, 1):
                    for b in range(BPC):
                        prev = (b - 1) % BPC
                        emit_loads((b + 1) % BPC, states[(b + 1) % BPC])
                        for m in range(NCHUNK):
                            emit_gram_strip(b, m, states[b])
                            emit_h0_chunk(prev, m, states[prev])
                        emit_tail(prev, states[prev])
            else:
                with tc.For_i(0, repeat, 1):
                    emit_all()

    nc.finalize()
    return nc


def _host_prep(x, u, W0, b0, W1, b1, include_xbc=False):
    """Build per-core input maps."""
    tnp = np.float16 if PRECISION == "f16" else np.float32
    gnp = tnp
    xT = np.ascontiguousarray(x.transpose(0, 2, 1)).astype(gnp)  # [B, 3, N]
    # [B, 128, 3N]: row d of x^T broadcast across the partition dim
    xbc = None
    if include_xbc:
        xbc = np.ascontiguousarray(
            np.broadcast_to(xT.reshape(B, 1, 3 * N), (B, 128, 3 * N))
        )
    norms = np.sqrt((x.astype(np.float64) ** 2).sum(-1)).astype(np.float32)  # [B, N]
    rhs2 = np.stack([norms, np.ones_like(norms)], axis=1)  # [B, 2, N]
    cb = (u @ W0[:G] + b0).astype(np.float32)  # [B, H]
    w0n = np.broadcast_to(W0[G], (B, H)).astype(np.float32)
    lw2 = np.ascontiguousarray(np.stack([w0n, cb], axis=1))  # [B, 2, H]
    w0d = np.ascontiguousarray(
        W0[G + 1 :].reshape(N // 128, 128, H).transpose(1, 0, 2).reshape(128, N // 128 * H)
    )

    # x rows chunked by 128: xr[b, p, 3m+d] = x[b, 128m+p, d]
    xr = np.ascontiguousarray(
        x.reshape(B, N // 128, 128, 3).transpose(0, 2, 1, 3)
        .reshape(B, 128, 3 * (N // 128))
    ).astype(tnp)

    in_maps = []
    for c in range(N_CORES):
        sl = slice(BPC * c, BPC * (c + 1))
        m = {
            "xT": np.ascontiguousarray(xT[sl]),
            "xr": np.ascontiguousarray(xr[sl]),
            "rhs2": np.ascontiguousarray(rhs2[sl]).astype(tnp),
            "lw2": np.ascontiguousarray(lw2[sl]).astype(tnp),
            "w0d": w0d.astype(tnp),
            "w1": np.ascontiguousarray(W1).astype(tnp),
            "b1t": np.tile(b1, N // H)[None, :].astype(tnp),
            "ones": np.ones((1, N), dtype=tnp),
        }
        if include_xbc:
            m["xbc"] = np.ascontiguousarray(xbc[sl])
        in_maps.append(m)
    return in_maps


def kernel(x, u, W0, b0, W1, b1, W2, b2, _run_kwargs=None):
    x = np.asarray(x, dtype=np.float32)
    u = np.asarray(u, dtype=np.float32)
    W0 = np.asarray(W0, dtype=np.float32)
    b0 = np.asarray(b0, dtype=np.float32)
    W1 = np.asarray(W1, dtype=np.float32)
    b1 = np.asarray(b1, dtype=np.float32)
    W2 = np.asarray(W2, dtype=np.float32)
    b2 = np.asarray(b2, dtype=np.float32)

    from concourse.bass_utils import run_bass_kernel_spmd

    with_b1 = bool(np.any(b1))
    key = ("nc", with_b1)
    if key not in _cached:
        _cached[key] = _build_nc(with_b1=with_b1)
    nc = _cached[key]

    in_maps = _host_prep(x, u, W0, b0, W1, b1)
    kw = dict(_run_kwargs or {})
    res = run_bass_kernel_spmd(nc, in_maps, list(range(N_CORES)), **kw)
    _cached["last_results"] = res
    y = np.concatenate([r["y"] for r in res.results], axis=0)  # [B,3,H] or [B,H,3]

    # host finish: out[b,o,d] = sum_h W2[h,o] y[b,d,h] / N + b2[o]*colsum_x[b,d]/N
    spec = "ho,bdh->bod" if y.shape[1] == 3 else "ho,bhd->bod"
    colsum = x.sum(axis=1)  # [B, 3]
    out = (
        np.einsum(spec, W2.astype(np.float64), y.astype(np.float64))
        + b2.astype(np.float64)[None, :, None] * colsum.astype(np.float64)[:, None, :]
    ) / N
    return out.astype(np.float32)


# revision 49
# speedup vs baseline: 1.1735x; 1.1735x over previous
"""Trainium2 Bass kernel for nn_NetworkLayer_79173427134941 (gnn_message_passing).

Reference computation (per batch item b, N=1024 points, 3D coords):
    norms = ||x_b||                      [N, 1]
    dots  = sqrt(x_b @ x_b^T)            [N, N]
    scalars = [u_b (G=8) | norms | dots] [N, 1033]
    h = LeakyReLU(scalars @ W0 + b0); h = LeakyReLU(h @ W1 + b1)
    fk = h @ W2 + b2                     [N, 128]
    out_b = einsum('io,id->od', fk, x_b) / N    [128, 3]

Strategy:
  - Data-parallel over batch: 4 batch items per core x 8 cores.
  - Never materialize dots in HBM: gram on TensorE, sqrt on ScalarE
    during PSUM->SBUF eviction, MLP fused on-chip in transposed [H, N] layout.
  - u-part + b0 folded into a host-precomputed K=2 rhs chunk [norms; ones].
  - LeakyReLU evictions fused to one DVE op each: leaky(x) = max(0.01x, x).
  - Final contraction y_b = x_b^T @ h1 runs as multiply-accumulate on the
    (otherwise idle) Pool engine; the last tiny [3,128]@[128,128] matmul +
    bias outer product runs on host:
       out_b^T = (x_b^T @ h1) @ W2 + b2 (x) colsum(x_b)
"""

import numpy as np

B, N, G = 32, 1024, 8
H, K_OUT = 128, 128
N_CORES = 8
BPC = B // N_CORES  # batch items per core

_cached = {}
PRECISION = "f16"


def _build_nc(precision=None, repeat=1, with_b1=True, fuse_leaky=False,
              y_engine="vector", interleave=True, leaky_wide=True,
              use_act_tail=False, pe_y=True, rotate=True, rowtile=False,
              wide_mm=False, abl=None):
    import concourse.tile as tile
    from concourse import bacc, mybir

    precision = precision or PRECISION
    f32 = mybir.dt.float32
    f32r = mybir.dt.float32r
    f16 = mybir.dt.float16
    tdt = f16 if precision == "f16" else f32r   # tail: h0/h1c/w1/xc
    mdt = f16 if precision == "f16" else f32r   # mid: dots/w0d
    MUL = mybir.AluOpType.mult
    ADD = mybir.AluOpType.add
    MAX = mybir.AluOpType.max

    nc = bacc.Bacc(
        "TRN2",
        target_bir_lowering=False,
        debug=False,
        enable_asserts=True,
        num_devices=N_CORES,
    )

    # DRAM I/O (per core)
    use_y_pre = abl not in ("no_y", "no_mlp", "empty")
    gdt = tdt
    xT_d = nc.dram_tensor("xT", [BPC, 3, N], gdt, kind="ExternalInput").ap()
    xr_d = None
    if use_y_pre and pe_y:
        xr_d = nc.dram_tensor("xr", [BPC, 128, 3 * (N // 128)], tdt,
                              kind="ExternalInput").ap()
    rhs2_d = nc.dram_tensor("rhs2", [BPC, 2, N], tdt, kind="ExternalInput").ap()
    lw2_d = nc.dram_tensor("lw2", [BPC, 2, H], tdt, kind="ExternalInput").ap()
    w0d_d = nc.dram_tensor("w0d", [128, 1024], mdt, kind="ExternalInput").ap()
    w1_d = nc.dram_tensor("w1", [128, H], tdt, kind="ExternalInput").ap()
    b1t_d = ones_d = None
    if with_b1:
        b1t_d = nc.dram_tensor("b1t", [1, N], tdt, kind="ExternalInput").ap()
        ones_d = nc.dram_tensor("ones", [1, N], tdt, kind="ExternalInput").ap()
    y_d = nc.dram_tensor("y", [BPC, H, 3], f32, kind="ExternalOutput").ap()

    NCHUNK = N // 128  # 8 K-chunks of the dots matmul
    use_y = abl not in ("no_y", "no_mlp", "empty")
    use_mlp = abl not in ("no_mlp", "empty")
    use_gram = abl != "empty"
    gram_strips = 1 if abl == "tiny_gram" else NCHUNK

    with tile.TileContext(nc) as tc:
        with (
            tc.tile_pool(name="const", bufs=1) as constp,
            tc.tile_pool(name="slots", bufs=1) as slotp,
            tc.tile_pool(name="data", bufs=2) as datap,
            tc.tile_pool(name="dots", bufs=2) as dotsp,
            tc.tile_pool(name="act", bufs=2) as actp,
            tc.tile_pool(name="yout", bufs=2) as youtp,
            tc.tile_pool(name="gram", bufs=2, space="PSUM") as gramp,
            tc.tile_pool(name="h0p", bufs=1, space="PSUM") as h0pp,
            tc.tile_pool(name="h1p", bufs=1, space="PSUM") as h1pp,
            tc.tile_pool(name="yp", bufs=1, space="PSUM") as ypp,
        ):
            # const tiles (DMAs issued after batch-0 loads; see emit_consts)
            w0d_sb = constp.tile([128, 1024], mdt)
            w1_sb = constp.tile([128, H], tdt)
            b1t_sb = ones_sb = None
            if with_b1:
                b1t_sb = constp.tile([1, N], tdt, name="b1t_sb")
                ones_sb = constp.tile([1, N], tdt, name="ones_sb")

            def emit_consts():
                if not use_mlp:
                    return
                nc.sync.dma_start(out=w0d_sb[:], in_=w0d_d[:])
                nc.sync.dma_start(out=w1_sb[:], in_=w1_d[:])
                if with_b1:
                    nc.sync.dma_start(out=b1t_sb[:], in_=b1t_d[:])
                    nc.sync.dma_start(out=ones_sb[:], in_=ones_d[:])

            def leaky_evict(out_ap, ps_ap, use_act=False):
                # leaky(x) = 0.01*x + 0.99*relu(x); two ops so each reads
                # PSUM once (walrus forbids two PSUM reads per instruction).
                ltmp = actp.tile(
                    [ps_ap.shape[0], ps_ap.shape[1]], f32, tag="ltmp", bufs=4
                )
                if use_act:
                    # Act has slack at the pipeline tail once sqrts drained
                    nc.scalar.activation(
                        ltmp[:], ps_ap, mybir.ActivationFunctionType.Relu,
                        bias=0.0, scale=0.99,
                    )
                else:
                    nc.vector.tensor_scalar(
                        ltmp[:], ps_ap, 0.0, 0.99, MAX, MUL
                    )
                nc.vector.scalar_tensor_tensor(
                    out_ap, ps_ap, 0.01, ltmp[:], MUL, ADD
                )

            def alloc_loads(b, st, persist=False):
                """Create the input + dots tiles for item b's slot."""
                P = slotp if persist else None
                sfx = str(b) if persist else ""

                def mk(pool, shape, dt_, tag):
                    pp = P if persist else pool
                    return pp.tile(shape, dt_, tag=tag + sfx, name=f"{tag}{b}")

                st["xT"] = mk(datap, [35, N] if rowtile else [3, N], gdt, "xT")
                if use_mlp:
                    st["rhs2"] = mk(datap, [2, N], tdt, "rhs2")
                    st["lw2"] = mk(datap, [2, H], tdt, "lw2")
                st["dots"] = mk(dotsp, [128, gram_strips * N], mdt, "dots")
                if use_y and pe_y:
                    # x chunk-rows [i%128, 3*(i//128)+d] for the PE y-reduce
                    st["xr"] = mk(datap, [128, 3 * NCHUNK], tdt, "xr")
                elif use_y:
                    # x^T rows, broadcast on-chip across partitions (Pool)
                    st["xf"] = mk(datap, [1, 3 * N], tdt, "xf")
                    st["xbc"] = mk(datap, [128, 3 * N], tdt, "xbc")

            def emit_loads(b, st):
                """Input DMAs for item b (prefetchable)."""
                if "xT" not in st:
                    alloc_loads(b, st)
                nc.sync.dma_start(out=st["xT"][0:3, :], in_=xT_d[b])
                if rowtile:
                    # second copy at partition 32 for PE row-group 1
                    nc.sync.dma_start(out=st["xT"][32:35, :], in_=xT_d[b])
                if use_mlp:
                    nc.sync.dma_start(out=st["rhs2"][:], in_=rhs2_d[b])
                    nc.sync.dma_start(out=st["lw2"][:], in_=lw2_d[b])
                if use_y and pe_y:
                    nc.sync.dma_start(out=st["xr"][:], in_=xr_d[b])
                elif use_y:
                    nc.sync.dma_start(out=st["xf"][:],
                                      in_=xT_d[b].flatten()[None, :])
                    nc.gpsimd.partition_broadcast(st["xbc"][:], st["xf"][:])

            def emit_gram_strip(b, m, st):
                """Gram strip m of batch b: 2 matmuls + sqrt eviction."""
                if m == 0 and "xT" not in st:
                    emit_loads(b, st)
                xT_sb, dots_sb = st["xT"], st["dots"]
                g_ps = gramp.tile([128, N], f32, tag="g", name=f"g{b}_{m}")
                # with rowtile, alternate strips between PE row groups 0/1
                # (base partition 0/32) so consecutive strips' matmuls run
                # concurrently in the array
                base = 32 * (m % 2) if rowtile else 0
                lhsT = xT_sb[base : base + 3, 128 * m : 128 * (m + 1)]
                if wide_mm:
                    nc.tensor.matmul(
                        g_ps[:], lhsT, xT_sb[base : base + 3, :],
                        start=True, stop=True,
                    )
                else:
                    for half in range(2):
                        nc.tensor.matmul(
                            g_ps[:, 512 * half : 512 * (half + 1)],
                            lhsT,
                            xT_sb[base : base + 3, 512 * half : 512 * (half + 1)],
                            start=True,
                            stop=True,
                        )
                nc.scalar.sqrt(dots_sb[:, N * m : N * (m + 1)], g_ps[:])

            def emit_h0_chunk(b, c, st):
                """Layer-0 K-chunk c of batch b (needs dots strip c only)."""
                if c == 0:
                    h0_ps = h0pp.tile([128, N], f32, tag="h0ps", name=f"h0ps{b}")
                    st["h0ps"] = h0_ps
                    if wide_mm:
                        nc.tensor.matmul(
                            h0_ps[:], st["lw2"][:], st["rhs2"][:],
                            start=True, stop=False,
                        )
                    else:
                        for half in range(2):
                            sl = slice(512 * half, 512 * (half + 1))
                            nc.tensor.matmul(
                                h0_ps[:, sl],
                                st["lw2"][:],
                                st["rhs2"][:, sl],
                                start=True,
                                stop=False,
                            )
                h0_ps, dots_sb = st["h0ps"], st["dots"]
                cc = min(c, gram_strips - 1)
                lhsT = w0d_sb[:, 128 * c : 128 * (c + 1)]
                if wide_mm:
                    nc.tensor.matmul(
                        h0_ps[:], lhsT, dots_sb[:, N * cc : N * (cc + 1)],
                        start=False, stop=(c == NCHUNK - 1),
                    )
                else:
                    for half in range(2):
                        nc.tensor.matmul(
                            h0_ps[:, 512 * half : 512 * (half + 1)],
                            lhsT,
                            dots_sb[:, N * cc + 512 * half : N * cc + 512 * (half + 1)],
                            start=False,
                            stop=(c == NCHUNK - 1),
                        )
                if c == NCHUNK - 1:
                    h0_sb = actp.tile([128, N], tdt, tag="h0", name=f"h0{b}")
                    st["h0"] = h0_sb
                    ua = use_act_tail and b == BPC - 1
                    if leaky_wide:
                        leaky_evict(h0_sb[:], h0_ps[:], use_act=ua)
                    else:
                        for half in range(2):
                            sl = slice(512 * half, 512 * (half + 1))
                            leaky_evict(h0_sb[:, sl], h0_ps[:, sl], use_act=ua)

            def emit_h1rev_group(b, st, mg):
                """4 j-blocks of layer 1 in [N, H] layout + leaky evict."""
                h0_sb = st["h0"]
                ua = use_act_tail and b == BPC - 1
                if mg == 0:
                    st["h1c"] = actp.tile([128, N], tdt, tag="h1c",
                                          name=f"h1c{b}")
                h1t_sb = st["h1c"]
                ps = h1pp.tile([128, 512], f32, tag="h1ps",
                               name=f"h1ps{b}_{mg}")
                for q in range(4):
                    m = 4 * mg + q
                    qsl = slice(128 * q, 128 * (q + 1))
                    if with_b1:
                        nc.tensor.matmul(
                            ps[:, qsl],
                            ones_sb[:, 0:128],
                            b1t_sb[:, 0:128],
                            start=True,
                            stop=False,
                        )
                    nc.tensor.matmul(
                        ps[:, qsl],
                        h0_sb[:, 128 * m : 128 * (m + 1)],
                        w1_sb[:],
                        start=not with_b1,
                        stop=True,
                    )
                leaky_evict(h1t_sb[:, 512 * mg : 512 * (mg + 1)], ps[:],
                            use_act=ua)

            def emit_y_group(b, st, mg):
                """y[h, d] = sum_i h1T[i, h] x[i, d]: lhsT = h1T block,
                rhs = x chunk -> only 3 output columns per matmul."""
                h1t_sb, xr_sb = st["h1c"], st["xr"]
                if mg == 0:
                    st["yps"] = ypp.tile([128, 4], f32, tag="yps",
                                         name=f"yps{b}")
                yT_ps = st["yps"]
                for q in range(4):
                    m = 4 * mg + q
                    nc.tensor.matmul(
                        yT_ps[:, 0:3],
                        h1t_sb[:, 128 * m : 128 * (m + 1)],
                        xr_sb[:, 3 * m : 3 * (m + 1)],
                        start=(m == 0),
                        stop=(m == NCHUNK - 1),
                    )
                if mg == 1:
                    yT_sb = youtp.tile([128, 4], f32, tag="y", name=f"y{b}")
                    nc.vector.tensor_scalar(yT_sb[:, 0:3], yT_ps[:, 0:3],
                                            1.0, None, MUL)
                    nc.sync.dma_start(out=y_d[b], in_=yT_sb[:, 0:3])

            def emit_tail_pe_y(b, st):
                for mg in range(2):
                    emit_h1rev_group(b, st, mg)
                for mg in range(2):
                    emit_y_group(b, st, mg)

            def emit_tail(b, st):
                """Layer 1 (transposed [H, N] layout) + output contraction."""
                if pe_y and use_y:
                    emit_tail_pe_y(b, st)
                    return
                h0_sb = st["h0"]
                h1_ps = h1pp.tile([128, N], f32, tag="h1ps", name=f"h1ps{b}")
                for half in range(2):
                    sl = slice(512 * half, 512 * (half + 1))
                    if with_b1:
                        nc.tensor.matmul(
                            h1_ps[:, sl],
                            b1t_sb[:, 0:128],
                            ones_sb[:, sl],
                            start=True,
                            stop=False,
                        )
                    nc.tensor.matmul(
                        h1_ps[:, sl],
                        w1_sb[:],
                        h0_sb[:, sl],
                        start=not with_b1,
                        stop=True,
                    )
                h1c_sb = actp.tile([128, N], tdt, tag="h1c", name=f"h1c{b}")
                ua = use_act_tail and b == BPC - 1
                if leaky_wide:
                    leaky_evict(h1c_sb[:], h1_ps[:], use_act=ua)
                else:
                    for half in range(2):
                        sl = slice(512 * half, 512 * (half + 1))
                        leaky_evict(h1c_sb[:, sl], h1_ps[:, sl], use_act=ua)

                yT_sb = youtp.tile([128, 4], f32, tag="y", name=f"y{b}")
                if use_y:
                    # y_b^T[h, d] = sum_i h1^T[h, i] * x[i, d]: free-axis
                    # multiply-reduce against the broadcast x rows
                    xbc_sb = st["xbc"]
                    yeng = nc.gpsimd if y_engine == "gpsimd" else nc.vector
                    for d in range(3):
                        ysc = actp.tile([128, N], tdt, tag="ysc",
                                        name=f"ysc{b}_{d}")
                        yeng.scalar_tensor_tensor(
                            ysc[:],
                            h1c_sb[:],
                            1.0,
                            xbc_sb[:, N * d : N * (d + 1)],
                            MUL,
                            MUL,
                            accum_out=yT_sb[:, d : d + 1],
                        )
                else:
                    nc.vector.tensor_scalar(
                        yT_sb[:, 0:3], h1c_sb[:, 0:3], 1.0, None, MUL
                    )
                nc.sync.dma_start(out=y_d[b], in_=yT_sb[:, 0:3])

            def emit_dummy_tail(b, st):
                """Ablation tails that keep the output DMA alive."""
                yT_sb = youtp.tile([128, 4], f32, tag="y", name=f"y{b}")
                if use_gram:
                    nc.vector.tensor_scalar(
                        yT_sb[:, 0:3], st["dots"][:, 0:3], 1.0, None, MUL
                    )
                else:
                    nc.vector.memset(yT_sb[:, 0:3], 0.0)
                nc.sync.dma_start(out=y_d[b], in_=yT_sb[:, 0:3])

            def emit_all():
                states = [dict() for _ in range(BPC)]
                if abl == "empty":
                    for b in range(BPC):
                        xT_sb = datap.tile([3, N], gdt, tag="xT", name=f"xT{b}")
                        nc.sync.dma_start(out=xT_sb[0:3, :], in_=xT_d[b])
                        states[b]["xT"] = xT_sb
                        emit_dummy_tail(b, states[b])
                    return
                if interleave and use_mlp and gram_strips == NCHUNK:
                    # Stage b emits gram strips of item b interleaved with h0
                    # chunks of item b-1, so the PE FIFO always has gram work
                    # to fill the sqrt-paced h0 stalls, and the sqrt stream of
                    # item b starts right behind item b-1's. Input DMAs are
                    # prefetched one stage ahead.
                    emit_loads(0, states[0])
                    emit_consts()
                    for b in range(BPC + 1):
                        if b + 1 < BPC:
                            emit_loads(b + 1, states[b + 1])
                        for m in range(NCHUNK):
                            if b < BPC:
                                emit_gram_strip(b, m, states[b])
                            if b >= 1:
                                emit_h0_chunk(b - 1, m, states[b - 1])
                        if b >= 1:
                            emit_tail(b - 1, states[b - 1])
                    return
                for b in range(BPC):
                    if b == 0:
                        emit_loads(0, states[0])
                        emit_consts()
                    for m in range(gram_strips):
                        emit_gram_strip(b, m, states[b])
                    if use_mlp:
                        if b >= 1:
                            emit_tail(b - 1, states[b - 1])
                        for c in range(NCHUNK):
                            emit_h0_chunk(b, c, states[b])
                    else:
                        emit_dummy_tail(b, states[b])
                if use_mlp:
                    emit_tail(BPC - 1, states[BPC - 1])

            can_rotate = rotate and interleave and use_mlp and gram_strips == NCHUNK
            if repeat == 1:
                emit_all()
            elif can_rotate:
                # Software-pipeline ACROSS loop iterations: each body stage
                # b runs gram(b) + h0(b-1 mod 4) + tail(b-1 mod 4); the
                # b=0 stage consumes slot 3 written by the previous
                # iteration (primed once by the prologue), so the sqrt
                # stream never drains at the loop back-edge.
                states = [dict() for _ in range(BPC)]
                for b in range(BPC):
                    alloc_loads(b, states[b], persist=True)
                emit_loads(BPC - 1, states[BPC - 1])
                emit_consts()  # loop-invariant: DMA'd once, outside the loop
                for m in range(NCHUNK):
                    emit_gram_strip(BPC - 1, m, states[BPC - 1])
                emit_loads(0, states[0])
                with tc.For_i(0, repeat, 1):
                    for b in range(BPC):
                        prev = (b - 1) % BPC
                        emit_loads((b + 1) % BPC, states[(b + 1) % BPC])
                        # h0 of prev first: its sqrt inputs landed last
                        # stage, so the PE FIFO never blocks on them; the
                        # eviction (chunk 7) then drains on DVE while the
                        # gram strips of b run.
                        for m in range(NCHUNK):
                            emit_h0_chunk(prev, m, states[prev])
                        # gram of b, with prev's tail matmuls sprinkled in
                        # late enough that their DVE evictions are done —
                        # they fill the sqrt-paced PSUM-recycle stalls.
                        for m in range(NCHUNK):
                            emit_gram_strip(b, m, states[b])
                            if pe_y:
                                if m == 3:
                                    emit_h1rev_group(prev, states[prev], 0)
                                elif m == 4:
                                    emit_h1rev_group(prev, states[prev], 1)
                                elif m == 5:
                                    emit_y_group(prev, states[prev], 0)
                                elif m == 6:
                                    emit_y_group(prev, states[prev], 1)
                        if not pe_y:
                            emit_tail(prev, states[prev])
            else:
                with tc.For_i(0, repeat, 1):
                    emit_all()

    nc.finalize()
    return nc


def _host_prep(x, u, W0, b0, W1, b1, include_xbc=False):
    """Build per-core input maps."""
    tnp = np.float16 if PRECISION == "f16" else np.float32
    gnp = tnp
    xT = np.ascontiguousarray(x.transpose(0, 2, 1)).astype(gnp)  # [B, 3, N]
    # [B, 128, 3N]: row d of x^T broadcast across the partition dim
    xbc = None
    if include_xbc:
        xbc = np.ascontiguousarray(
            np.broadcast_to(xT.reshape(B, 1, 3 * N), (B, 128, 3 * N))
        )
    norms = np.sqrt((x.astype(np.float64) ** 2).sum(-1)).astype(np.float32)  # [B, N]
    rhs2 = np.stack([norms, np.ones_like(norms)], axis=1)  # [B, 2, N]
    cb = (u @ W0[:G] + b0).astype(np.float32)  # [B, H]
    w0n = np.broadcast_to(W0[G], (B, H)).astype(np.float32)
    lw2 = np.ascontiguousarray(np.stack([w0n, cb], axis=1))  # [B, 2, H]
    w0d = np.ascontiguousarray(
        W0[G + 1 :].reshape(N // 128, 128, H).transpose(1, 0, 2).reshape(128, N // 128 * H)
    )

    # x rows chunked by 128: xr[b, p, 3m+d] = x[b, 128m+p, d]
    xr = np.ascontiguousarray(
        x.reshape(B, N // 128, 128, 3).transpose(0, 2, 1, 3)
        .reshape(B, 128, 3 * (N // 128))
    ).astype(tnp)

    in_maps = []
    for c in range(N_CORES):
        sl = slice(BPC * c, BPC * (c + 1))
        m = {
            "xT": np.ascontiguousarray(xT[sl]),
            "xr": np.ascontiguousarray(xr[sl]),
            "rhs2": np.ascontiguousarray(rhs2[sl]).astype(tnp),
            "lw2": np.ascontiguousarray(lw2[sl]).astype(tnp),
            "w0d": w0d.astype(tnp),
            "w1": np.ascontiguousarray(W1).astype(tnp),
            "b1t": np.tile(b1, N // H)[None, :].astype(tnp),
            "ones": np.ones((1, N), dtype=tnp),
        }
        if include_xbc:
            m["xbc"] = np.ascontiguousarray(xbc[sl])
        in_maps.append(m)
    return in_maps


def kernel(x, u, W0, b0, W1, b1, W2, b2, _run_kwargs=None):
    x = np.asarray(x, dtype=np.float32)
    u = np.asarray(u, dtype=np.float32)
    W0 = np.asarray(W0, dtype=np.float32)
    b0 = np.asarray(b0, dtype=np.float32)
    W1 = np.asarray(W1, dtype=np.float32)
    b1 = np.asarray(b1, dtype=np.float32)
    W2 = np.asarray(W2, dtype=np.float32)
    b2 = np.asarray(b2, dtype=np.float32)

    from concourse.bass_utils import run_bass_kernel_spmd

    with_b1 = bool(np.any(b1))
    key = ("nc", with_b1)
    if key not in _cached:
        _cached[key] = _build_nc(with_b1=with_b1)
    nc = _cached[key]

    in_maps = _host_prep(x, u, W0, b0, W1, b1)
    kw = dict(_run_kwargs or {})
    res = run_bass_kernel_spmd(nc, in_maps, list(range(N_CORES)), **kw)
    _cached["last_results"] = res
    y = np.concatenate([r["y"] for r in res.results], axis=0)  # [B,3,H] or [B,H,3]

    # host finish: out[b,o,d] = sum_h W2[h,o] y[b,d,h] / N + b2[o]*colsum_x[b,d]/N
    spec = "ho,bdh->bod" if y.shape[1] == 3 else "ho,bhd->bod"
    colsum = x.sum(axis=1)  # [B, 3]
    out = (
        np.einsum(spec, W2.astype(np.float64), y.astype(np.float64))
        + b2.astype(np.float64)[None, :, None] * colsum.astype(np.float64)[:, None, :]
    ) / N
    return out.astype(np.float32)


# revision 61
# speedup vs baseline: 1.2606x; 1.0742x over previous
"""Trainium2 Bass kernel for nn_NetworkLayer_79173427134941 (gnn_message_passing).

Reference computation (per batch item b, N=1024 points, 3D coords):
    norms = ||x_b||                      [N, 1]
    dots  = sqrt(x_b @ x_b^T)            [N, N]
    scalars = [u_b (G=8) | norms | dots] [N, 1033]
    h = LeakyReLU(scalars @ W0 + b0); h = LeakyReLU(h @ W1 + b1)
    fk = h @ W2 + b2                     [N, 128]
    out_b = einsum('io,id->od', fk, x_b) / N    [128, 3]

Strategy:
  - Data-parallel over batch: 4 batch items per core x 8 cores.
  - Never materialize dots in HBM: gram on TensorE, sqrt on ScalarE
    during PSUM->SBUF eviction, MLP fused on-chip in transposed [H, N] layout.
  - u-part + b0 folded into a host-precomputed K=2 rhs chunk [norms; ones].
  - LeakyReLU evictions fused to one DVE op each: leaky(x) = max(0.01x, x).
  - Final contraction y_b = x_b^T @ h1 runs as multiply-accumulate on the
    (otherwise idle) Pool engine; the last tiny [3,128]@[128,128] matmul +
    bias outer product runs on host:
       out_b^T = (x_b^T @ h1) @ W2 + b2 (x) colsum(x_b)
"""

import numpy as np

B, N, G = 32, 1024, 8
H, K_OUT = 128, 128
N_CORES = 8
BPC = B // N_CORES  # batch items per core

_cached = {}
PRECISION = "f16"


def _build_nc(precision=None, repeat=1, with_b1=True, fuse_leaky=False,
              y_engine="vector", interleave=True, leaky_wide=True,
              use_act_tail=False, pe_y=True, rotate=True, rowtile=False,
              wide_mm=False, unroll=False, gram0_early=False, fold_rhs2=True,
              abl=None):
    import concourse.tile as tile
    from concourse import bacc, mybir

    precision = precision or PRECISION
    f32 = mybir.dt.float32
    f32r = mybir.dt.float32r
    f16 = mybir.dt.float16
    tdt = f16 if precision == "f16" else f32r   # tail: h0/h1c/w1/xc
    mdt = f16 if precision == "f16" else f32r   # mid: dots/w0d
    MUL = mybir.AluOpType.mult
    ADD = mybir.AluOpType.add
    MAX = mybir.AluOpType.max

    nc = bacc.Bacc(
        "TRN2",
        target_bir_lowering=False,
        debug=False,
        enable_asserts=True,
        num_devices=N_CORES,
    )

    # DRAM I/O (per core)
    use_y_pre = abl not in ("no_y", "no_mlp", "empty")
    gdt = tdt
    xT_d = nc.dram_tensor("xT", [BPC, 3, N], gdt, kind="ExternalInput").ap()
    xr_d = None
    if use_y_pre and pe_y:
        xr_d = nc.dram_tensor("xr", [BPC, 128, 3 * (N // 128)], tdt,
                              kind="ExternalInput").ap()
    rhs2_d = nc.dram_tensor("rhs2", [BPC, 2, N], tdt, kind="ExternalInput").ap()
    lw2_d = nc.dram_tensor("lw2", [BPC, 2, H], tdt, kind="ExternalInput").ap()
    cw_d = e1r_d = None
    if fold_rhs2:
        cw_d = nc.dram_tensor("cw", [BPC, 128, 2], f32, kind="ExternalInput").ap()
        e1r_d = nc.dram_tensor("e1r", [BPC, 1, 512], tdt, kind="ExternalInput").ap()
    w0d_d = nc.dram_tensor("w0d", [128, 1024], mdt, kind="ExternalInput").ap()
    w1_d = nc.dram_tensor("w1", [128, H], tdt, kind="ExternalInput").ap()
    b1t_d = ones_d = None
    if with_b1:
        b1t_d = nc.dram_tensor("b1t", [1, N], tdt, kind="ExternalInput").ap()
        ones_d = nc.dram_tensor("ones", [1, N], tdt, kind="ExternalInput").ap()
    y_d = nc.dram_tensor("y", [BPC, H, 3], f32, kind="ExternalOutput").ap()

    NCHUNK = N // 128  # 8 K-chunks of the dots matmul
    use_y = abl not in ("no_y", "no_mlp", "empty")
    use_mlp = abl not in ("no_mlp", "empty")
    use_gram = abl != "empty"
    gram_strips = 1 if abl == "tiny_gram" else NCHUNK

    with tile.TileContext(nc) as tc:
        with (
            tc.tile_pool(name="const", bufs=1) as constp,
            tc.tile_pool(name="slots", bufs=1) as slotp,
            tc.tile_pool(name="data", bufs=2) as datap,
            tc.tile_pool(name="dots", bufs=2) as dotsp,
            tc.tile_pool(name="act", bufs=2) as actp,
            tc.tile_pool(name="yout", bufs=2) as youtp,
            tc.tile_pool(name="gram", bufs=2, space="PSUM") as gramp,
            tc.tile_pool(name="h0p", bufs=1, space="PSUM") as h0pp,
            tc.tile_pool(name="h1p", bufs=1, space="PSUM") as h1pp,
            tc.tile_pool(name="yp", bufs=1, space="PSUM") as ypp,
        ):
            # const tiles (DMAs issued after batch-0 loads; see emit_consts)
            w0d_sb = constp.tile([128, 1024], mdt)
            w1_sb = constp.tile([128, H], tdt)
            b1t_sb = ones_sb = None
            if with_b1:
                b1t_sb = constp.tile([1, N], tdt, name="b1t_sb")
                ones_sb = constp.tile([1, N], tdt, name="ones_sb")

            def emit_consts():
                if not use_mlp:
                    return
                nc.sync.dma_start(out=w0d_sb[:], in_=w0d_d[:])
                nc.sync.dma_start(out=w1_sb[:], in_=w1_d[:])
                if with_b1:
                    nc.sync.dma_start(out=b1t_sb[:], in_=b1t_d[:])
                    nc.sync.dma_start(out=ones_sb[:], in_=ones_d[:])

            def leaky_evict(out_ap, ps_ap, use_act=False):
                # leaky(x) = 0.01*x + 0.99*relu(x); two ops so each reads
                # PSUM once (walrus forbids two PSUM reads per instruction).
                ltmp = actp.tile(
                    [ps_ap.shape[0], ps_ap.shape[1]], f32, tag="ltmp", bufs=4
                )
                if use_act:
                    # Act has slack at the pipeline tail once sqrts drained
                    nc.scalar.activation(
                        ltmp[:], ps_ap, mybir.ActivationFunctionType.Relu,
                        bias=0.0, scale=0.99,
                    )
                else:
                    nc.vector.tensor_scalar(
                        ltmp[:], ps_ap, 0.0, 0.99, MAX, MUL
                    )
                nc.vector.scalar_tensor_tensor(
                    out_ap, ps_ap, 0.01, ltmp[:], MUL, ADD
                )

            def alloc_loads(b, st, persist=False):
                """Create the input + dots tiles for item b's slot."""
                P = slotp if persist else None
                sfx = str(b) if persist else ""

                def mk(pool, shape, dt_, tag):
                    pp = P if persist else pool
                    return pp.tile(shape, dt_, tag=tag + sfx, name=f"{tag}{b}")

                st["xT"] = mk(datap, [35, N] if rowtile else [3, N], gdt, "xT")
                if use_mlp:
                    st["rhs2"] = mk(datap, [2, N], tdt, "rhs2")
                    if fold_rhs2:
                        st["cw"] = mk(datap, [128, 2], f32, "cw")
                        st["e1r"] = mk(datap, [1, 512], tdt, "e1r")
                        st["nbc"] = mk(datap, [128, N], tdt, "nbc")
                        st["e1bc"] = mk(datap, [128, 512], tdt, "e1bc")
                        st["zt"] = mk(datap, [128, N], f32, "zt")
                        st["zt1"] = mk(datap, [128, 512], f32, "zt1")
                    else:
                        st["lw2"] = mk(datap, [2, H], tdt, "lw2")
                st["dots"] = mk(dotsp, [128, gram_strips * N], mdt, "dots")
                if use_y and pe_y:
                    # x chunk-rows [i%128, 3*(i//128)+d] for the PE y-reduce
                    st["xr"] = mk(datap, [128, 3 * NCHUNK], tdt, "xr")
                elif use_y:
                    # x^T rows, broadcast on-chip across partitions (Pool)
                    st["xf"] = mk(datap, [1, 3 * N], tdt, "xf")
                    st["xbc"] = mk(datap, [128, 3 * N], tdt, "xbc")

            def emit_loads(b, st):
                """Input DMAs for item b (prefetchable)."""
                if "xT" not in st:
                    alloc_loads(b, st)
                nc.sync.dma_start(out=st["xT"][0:3, :], in_=xT_d[b])
                if rowtile:
                    # second copy at partition 32 for PE row-group 1
                    nc.sync.dma_start(out=st["xT"][32:35, :], in_=xT_d[b])
                if use_mlp:
                    nc.sync.dma_start(out=st["rhs2"][:], in_=rhs2_d[b])
                    if fold_rhs2:
                        nc.sync.dma_start(out=st["cw"][:], in_=cw_d[b])
                        nc.sync.dma_start(out=st["e1r"][:], in_=e1r_d[b])
                        nc.gpsimd.partition_broadcast(st["nbc"][:],
                                                      st["rhs2"][0:1, :])
                        nc.gpsimd.partition_broadcast(st["e1bc"][:],
                                                      st["e1r"][:])
                    else:
                        nc.sync.dma_start(out=st["lw2"][:], in_=lw2_d[b])
                if use_y and pe_y:
                    nc.sync.dma_start(out=st["xr"][:], in_=xr_d[b])
                elif use_y:
                    nc.sync.dma_start(out=st["xf"][:],
                                      in_=xT_d[b].flatten()[None, :])
                    nc.gpsimd.partition_broadcast(st["xbc"][:], st["xf"][:])

            def emit_gram_strip(b, m, st):
                """Gram strip m of batch b: 2 matmuls + sqrt eviction."""
                if m == 0 and "xT" not in st:
                    emit_loads(b, st)
                xT_sb, dots_sb = st["xT"], st["dots"]
                g_ps = gramp.tile([128, N], f32, tag="g", name=f"g{b}_{m}")
                # with rowtile, alternate strips between PE row groups 0/1
                # (base partition 0/32) so consecutive strips' matmuls run
                # concurrently in the array
                base = 32 * (m % 2) if rowtile else 0
                lhsT = xT_sb[base : base + 3, 128 * m : 128 * (m + 1)]
                if wide_mm:
                    nc.tensor.matmul(
                        g_ps[:], lhsT, xT_sb[base : base + 3, :],
                        start=True, stop=True,
                    )
                else:
                    for half in range(2):
                        nc.tensor.matmul(
                            g_ps[:, 512 * half : 512 * (half + 1)],
                            lhsT,
                            xT_sb[base : base + 3, 512 * half : 512 * (half + 1)],
                            start=True,
                            stop=True,
                        )
                nc.scalar.sqrt(dots_sb[:, N * m : N * (m + 1)], g_ps[:])

            def emit_h0_chunk(b, c, st):
                """Layer-0 K-chunk c of batch b (needs dots strip c only)."""
                if c == 0:
                    h0_ps = h0pp.tile([128, N], f32, tag="h0ps", name=f"h0ps{b}")
                    st["h0ps"] = h0_ps
                    if not fold_rhs2:
                        for half in range(2):
                            sl = slice(512 * half, 512 * (half + 1))
                            nc.tensor.matmul(
                                h0_ps[:, sl],
                                st["lw2"][:],
                                st["rhs2"][:, sl],
                                start=True,
                                stop=False,
                            )
                h0_ps, dots_sb = st["h0ps"], st["dots"]
                cc = min(c, gram_strips - 1)
                lhsT = w0d_sb[:, 128 * c : 128 * (c + 1)]
                for half in range(2):
                    nc.tensor.matmul(
                        h0_ps[:, 512 * half : 512 * (half + 1)],
                        lhsT,
                        dots_sb[:, N * cc + 512 * half : N * cc + 512 * (half + 1)],
                        start=(fold_rhs2 and c == 0),
                        stop=(c == NCHUNK - 1),
                    )
                if c == NCHUNK - 1:
                    h0_sb = actp.tile([128, N], tdt, tag="h0", name=f"h0{b}")
                    st["h0"] = h0_sb
                    ua = use_act_tail and b == BPC - 1
                    if fold_rhs2:
                        # z = ps + w0n[o]*norm_j + cb[o]; store
                        # h0' = relu(z) + (0.01/0.99)*(z - cb)  (the cb
                        # residue is corrected via e1 in the h1 eviction;
                        # the 0.99 factor is folded into W1 on the host)
                        zt, cw = st["zt"], st["cw"]
                        nc.vector.scalar_tensor_tensor(
                            zt[:], st["nbc"][:], cw[:, 1:2], h0_ps[:],
                            MUL, ADD,
                        )
                        ltmp = actp.tile([128, N], f32, tag="ltmp", bufs=4)
                        nc.vector.tensor_scalar(
                            ltmp[:], zt[:], cw[:, 0:1], 0.0, ADD, MAX
                        )
                        nc.vector.scalar_tensor_tensor(
                            h0_sb[:], zt[:], 0.01 / 0.99, ltmp[:], MUL, ADD
                        )
                    elif leaky_wide:
                        leaky_evict(h0_sb[:], h0_ps[:], use_act=ua)
                    else:
                        for half in range(2):
                            sl = slice(512 * half, 512 * (half + 1))
                            leaky_evict(h0_sb[:, sl], h0_ps[:, sl], use_act=ua)

            def emit_h1rev_group(b, st, mg):
                """4 j-blocks of layer 1 in [N, H] layout + leaky evict."""
                h0_sb = st["h0"]
                ua = use_act_tail and b == BPC - 1
                if mg == 0:
                    st["h1c"] = actp.tile([128, N], tdt, tag="h1c",
                                          name=f"h1c{b}")
                h1t_sb = st["h1c"]
                ps = h1pp.tile([128, 512], f32, tag="h1ps",
                               name=f"h1ps{b}_{mg}")
                for q in range(4):
                    m = 4 * mg + q
                    qsl = slice(128 * q, 128 * (q + 1))
                    if with_b1:
                        nc.tensor.matmul(
                            ps[:, qsl],
                            ones_sb[:, 0:128],
                            b1t_sb[:, 0:128],
                            start=True,
                            stop=False,
                        )
                    nc.tensor.matmul(
                        ps[:, qsl],
                        h0_sb[:, 128 * m : 128 * (m + 1)],
                        w1_sb[:],
                        start=not with_b1,
                        stop=True,
                    )
                if fold_rhs2:
                    # correct the cb residue: z1 = ps + e1 (e1 = 0.01*cb@W1,
                    # tiled per j-block, broadcast across partitions)
                    zt1 = st["zt1"]
                    nc.vector.scalar_tensor_tensor(
                        zt1[:], st["e1bc"][:], 1.0, ps[:], MUL, ADD
                    )
                    leaky_evict(h1t_sb[:, 512 * mg : 512 * (mg + 1)], zt1[:],
                                use_act=ua)
                else:
                    leaky_evict(h1t_sb[:, 512 * mg : 512 * (mg + 1)], ps[:],
                                use_act=ua)

            def emit_y_group(b, st, mg):
                """y[h, d] = sum_i h1T[i, h] x[i, d]: lhsT = h1T block,
                rhs = x chunk -> only 3 output columns per matmul."""
                h1t_sb, xr_sb = st["h1c"], st["xr"]
                if mg == 0:
                    st["yps"] = ypp.tile([128, 4], f32, tag="yps",
                                         name=f"yps{b}")
                yT_ps = st["yps"]
                for q in range(4):
                    m = 4 * mg + q
                    nc.tensor.matmul(
                        yT_ps[:, 0:3],
                        h1t_sb[:, 128 * m : 128 * (m + 1)],
                        xr_sb[:, 3 * m : 3 * (m + 1)],
                        start=(m == 0),
                        stop=(m == NCHUNK - 1),
                    )
                if mg == 1:
                    yT_sb = youtp.tile([128, 4], f32, tag="y", name=f"y{b}")
                    nc.vector.tensor_scalar(yT_sb[:, 0:3], yT_ps[:, 0:3],
                                            1.0, None, MUL)
                    nc.sync.dma_start(out=y_d[b], in_=yT_sb[:, 0:3])

            def emit_tail_pe_y(b, st):
                for mg in range(2):
                    emit_h1rev_group(b, st, mg)
                for mg in range(2):
                    emit_y_group(b, st, mg)

            def emit_tail(b, st):
                """Layer 1 (transposed [H, N] layout) + output contraction."""
                if pe_y and use_y:
                    emit_tail_pe_y(b, st)
                    return
                h0_sb = st["h0"]
                h1_ps = h1pp.tile([128, N], f32, tag="h1ps", name=f"h1ps{b}")
                for half in range(2):
                    sl = slice(512 * half, 512 * (half + 1))
                    if with_b1:
                        nc.tensor.matmul(
                            h1_ps[:, sl],
                            b1t_sb[:, 0:128],
                            ones_sb[:, sl],
                            start=True,
                            stop=False,
                        )
                    nc.tensor.matmul(
                        h1_ps[:, sl],
                        w1_sb[:],
                        h0_sb[:, sl],
                        start=not with_b1,
                        stop=True,
                    )
                h1c_sb = actp.tile([128, N], tdt, tag="h1c", name=f"h1c{b}")
                ua = use_act_tail and b == BPC - 1
                if leaky_wide:
                    leaky_evict(h1c_sb[:], h1_ps[:], use_act=ua)
                else:
                    for half in range(2):
                        sl = slice(512 * half, 512 * (half + 1))
                        leaky_evict(h1c_sb[:, sl], h1_ps[:, sl], use_act=ua)

                yT_sb = youtp.tile([128, 4], f32, tag="y", name=f"y{b}")
                if use_y:
                    # y_b^T[h, d] = sum_i h1^T[h, i] * x[i, d]: free-axis
                    # multiply-reduce against the broadcast x rows
                    xbc_sb = st["xbc"]
                    yeng = nc.gpsimd if y_engine == "gpsimd" else nc.vector
                    for d in range(3):
                        ysc = actp.tile([128, N], tdt, tag="ysc",
                                        name=f"ysc{b}_{d}")
                        yeng.scalar_tensor_tensor(
                            ysc[:],
                            h1c_sb[:],
                            1.0,
                            xbc_sb[:, N * d : N * (d + 1)],
                            MUL,
                            MUL,
                            accum_out=yT_sb[:, d : d + 1],
                        )
                else:
                    nc.vector.tensor_scalar(
                        yT_sb[:, 0:3], h1c_sb[:, 0:3], 1.0, None, MUL
                    )
                nc.sync.dma_start(out=y_d[b], in_=yT_sb[:, 0:3])

            def emit_dummy_tail(b, st):
                """Ablation tails that keep the output DMA alive."""
                yT_sb = youtp.tile([128, 4], f32, tag="y", name=f"y{b}")
                if use_gram:
                    nc.vector.tensor_scalar(
                        yT_sb[:, 0:3], st["dots"][:, 0:3], 1.0, None, MUL
                    )
                else:
                    nc.vector.memset(yT_sb[:, 0:3], 0.0)
                nc.sync.dma_start(out=y_d[b], in_=yT_sb[:, 0:3])

            def emit_all():
                states = [dict() for _ in range(BPC)]
                if abl == "empty":
                    for b in range(BPC):
                        xT_sb = datap.tile([3, N], gdt, tag="xT", name=f"xT{b}")
                        nc.sync.dma_start(out=xT_sb[0:3, :], in_=xT_d[b])
                        states[b]["xT"] = xT_sb
                        emit_dummy_tail(b, states[b])
                    return
                if interleave and use_mlp and gram_strips == NCHUNK:
                    # Stage b emits gram strips of item b interleaved with h0
                    # chunks of item b-1, so the PE FIFO always has gram work
                    # to fill the sqrt-paced h0 stalls, and the sqrt stream of
                    # item b starts right behind item b-1's. Input DMAs are
                    # prefetched one stage ahead.
                    emit_loads(0, states[0])
                    emit_consts()
                    for b in range(BPC + 1):
                        if b + 1 < BPC:
                            emit_loads(b + 1, states[b + 1])
                        for m in range(NCHUNK):
                            if b < BPC:
                                emit_gram_strip(b, m, states[b])
                            if b >= 1:
                                emit_h0_chunk(b - 1, m, states[b - 1])
                        if b >= 1:
                            emit_tail(b - 1, states[b - 1])
                    return
                for b in range(BPC):
                    if b == 0:
                        emit_loads(0, states[0])
                        emit_consts()
                    for m in range(gram_strips):
                        emit_gram_strip(b, m, states[b])
                    if use_mlp:
                        if b >= 1:
                            emit_tail(b - 1, states[b - 1])
                        for c in range(NCHUNK):
                            emit_h0_chunk(b, c, states[b])
                    else:
                        emit_dummy_tail(b, states[b])
                if use_mlp:
                    emit_tail(BPC - 1, states[BPC - 1])

            can_rotate = rotate and interleave and use_mlp and gram_strips == NCHUNK
            if repeat == 1:
                emit_all()
            elif can_rotate:
                # Software-pipeline ACROSS loop iterations: each body stage
                # b runs gram(b) + h0(b-1 mod 4) + tail(b-1 mod 4); the
                # b=0 stage consumes slot 3 written by the previous
                # iteration (primed once by the prologue), so the sqrt
                # stream never drains at the loop back-edge.
                states = [dict() for _ in range(BPC)]
                for b in range(BPC):
                    alloc_loads(b, states[b], persist=True)
                emit_loads(BPC - 1, states[BPC - 1])
                emit_consts()  # loop-invariant: DMA'd once, outside the loop
                for m in range(NCHUNK):
                    emit_gram_strip(BPC - 1, m, states[BPC - 1])
                emit_loads(0, states[0])

                def emit_stage(b):
                    prev = (b - 1) % BPC
                    emit_loads((b + 1) % BPC, states[(b + 1) % BPC])
                    g0 = 1 if gram0_early else 0
                    if gram0_early:
                        # give the sqrt stream a head start before the
                        # h0 block occupies the PE
                        emit_gram_strip(b, 0, states[b])
                    # h0 of prev first: its sqrt inputs landed last
                    # stage, so the PE FIFO never blocks on them; the
                    # eviction (chunk 7) then drains on DVE while the
                    # gram strips of b run.
                    for m in range(NCHUNK):
                        emit_h0_chunk(prev, m, states[prev])
                    # gram of b, with prev's tail matmuls sprinkled in
                    # late enough that their DVE evictions are done —
                    # they fill the sqrt-paced PSUM-recycle stalls.
                    for m in range(g0, NCHUNK):
                        emit_gram_strip(b, m, states[b])
                        if pe_y:
                            if m == 3:
                                emit_h1rev_group(prev, states[prev], 0)
                            elif m == 4:
                                emit_h1rev_group(prev, states[prev], 1)
                            elif m == 5:
                                emit_y_group(prev, states[prev], 0)
                            elif m == 6:
                                emit_y_group(prev, states[prev], 1)
                    if not pe_y:
                        emit_tail(prev, states[prev])

                if unroll:
                    for _ in range(repeat):
                        for b in range(BPC):
                            emit_stage(b)
                else:
                    with tc.For_i(0, repeat, 1):
                        for b in range(BPC):
                            emit_stage(b)
            else:
                with tc.For_i(0, repeat, 1):
                    emit_all()

    nc.finalize()
    return nc


def _host_prep(x, u, W0, b0, W1, b1, include_xbc=False, fold_rhs2=True):
    """Build per-core input maps."""
    tnp = np.float16 if PRECISION == "f16" else np.float32
    gnp = tnp
    xT = np.ascontiguousarray(x.transpose(0, 2, 1)).astype(gnp)  # [B, 3, N]
    # [B, 128, 3N]: row d of x^T broadcast across the partition dim
    xbc = None
    if include_xbc:
        xbc = np.ascontiguousarray(
            np.broadcast_to(xT.reshape(B, 1, 3 * N), (B, 128, 3 * N))
        )
    norms = np.sqrt((x.astype(np.float64) ** 2).sum(-1)).astype(np.float32)  # [B, N]
    rhs2 = np.stack([norms, np.ones_like(norms)], axis=1)  # [B, 2, N]
    cb = (u @ W0[:G] + b0).astype(np.float32)  # [B, H]
    w0n = np.broadcast_to(W0[G], (B, H)).astype(np.float32)
    lw2 = np.ascontiguousarray(np.stack([w0n, cb], axis=1))  # [B, 2, H]
    w0d = np.ascontiguousarray(
        W0[G + 1 :].reshape(N // 128, 128, H).transpose(1, 0, 2).reshape(128, N // 128 * H)
    )

    # x rows chunked by 128: xr[b, p, 3m+d] = x[b, 128m+p, d]
    xr = np.ascontiguousarray(
        x.reshape(B, N // 128, 128, 3).transpose(0, 2, 1, 3)
        .reshape(B, 128, 3 * (N // 128))
    ).astype(tnp)

    in_maps = []
    for c in range(N_CORES):
        sl = slice(BPC * c, BPC * (c + 1))
        m = {
            "xT": np.ascontiguousarray(xT[sl]),
            "xr": np.ascontiguousarray(xr[sl]),
            "rhs2": np.ascontiguousarray(rhs2[sl]).astype(tnp),
            "lw2": np.ascontiguousarray(lw2[sl]).astype(tnp),
            "w0d": w0d.astype(tnp),
            "w1": np.ascontiguousarray(W1).astype(tnp),
            "b1t": np.tile(b1, N // H)[None, :].astype(tnp),
            "ones": np.ones((1, N), dtype=tnp),
        }
        if fold_rhs2:
            # leaky's 0.99 factor folded into W1; h0 stores
            # relu(z) + (0.01/0.99)z with the cb part of z corrected via e1
            m["w1"] = np.ascontiguousarray(0.99 * W1).astype(tnp)
            cw = np.stack([cb[sl], np.broadcast_to(W0[G], (BPC, H))], axis=2)
            m["cw"] = np.ascontiguousarray(cw).astype(np.float32)
            e1 = 0.01 * (cb[sl] @ W1)  # [BPC, H]
            m["e1r"] = np.ascontiguousarray(
                np.tile(e1, (1, 4))[:, None, :]
            ).astype(tnp)
        if include_xbc:
            m["xbc"] = np.ascontiguousarray(xbc[sl])
        in_maps.append(m)
    return in_maps


def kernel(x, u, W0, b0, W1, b1, W2, b2, _run_kwargs=None):
    x = np.asarray(x, dtype=np.float32)
    u = np.asarray(u, dtype=np.float32)
    W0 = np.asarray(W0, dtype=np.float32)
    b0 = np.asarray(b0, dtype=np.float32)
    W1 = np.asarray(W1, dtype=np.float32)
    b1 = np.asarray(b1, dtype=np.float32)
    W2 = np.asarray(W2, dtype=np.float32)
    b2 = np.asarray(b2, dtype=np.float32)

    from concourse.bass_utils import run_bass_kernel_spmd

    with_b1 = bool(np.any(b1))
    key = ("nc", with_b1)
    if key not in _cached:
        _cached[key] = _build_nc(with_b1=with_b1)
    nc = _cached[key]

    in_maps = _host_prep(x, u, W0, b0, W1, b1)
    kw = dict(_run_kwargs or {})
    res = run_bass_kernel_spmd(nc, in_maps, list(range(N_CORES)), **kw)
    _cached["last_results"] = res
    y = np.concatenate([r["y"] for r in res.results], axis=0)  # [B,3,H] or [B,H,3]

    # host finish: out[b,o,d] = sum_h W2[h,o] y[b,d,h] / N + b2[o]*colsum_x[b,d]/N
    spec = "ho,bdh->bod" if y.shape[1] == 3 else "ho,bhd->bod"
    colsum = x.sum(axis=1)  # [B, 3]
    out = (
        np.einsum(spec, W2.astype(np.float64), y.astype(np.float64))
        + b2.astype(np.float64)[None, :, None] * colsum.astype(np.float64)[:, None, :]
    ) / N
    return out.astype(np.float32)
